# revision 1
# baseline (speedup 1.0000x reference)
"""Trainium2 Bass kernel for nn_Loss2D_57432302682561.

Math per view v (V = 40000 views, 68 landmarks each):
    y block  = points_y[68 + 68v : 68 + 68(v+1)]          # [68, 15]
    pt       = y[:, 0:2]                                   # target 2D points
    scale    = y[0, 2];  R = y[0, 3:12].reshape(3,3);  t = y[0, 12:15]
    M        = inv(scale * R) = adj(R) / (scale * det(R))  # [3, 3]
    proj     = (points_x - t) @ M  -> first 2 cols         # [68, 2]
    mask     = (pt[:,0] >= 0) | (pt[:,1] >= 0)
    dist     = sqrt(sum((pt - proj)^2, -1))
    loss_v   = sum(dist * mask) / sum(mask)
    out      = sum_v loss_v / V^2

Strategy (8 NeuronCores, data-parallel over views):
  - Each core gets 5000 contiguous view blocks (20.4 MB) - DMA'd in big
    contiguous chunks into SBUF with 128 views on partitions, the full
    1020-float view row contiguous in the free dim (full-bandwidth DMA).
  - 3x3 inverse math is batched across views with views on partitions and
    tiles along the free dim (41 small DVE ops per 10-tile batch).
  - The projection (points_x - t) @ M collapses to ONE tiny-K PE matmul
    per 128-view tile: weights = per-view [M00,M10,M20,c0,M01,M11,M21,c1]
    rows (built via one PE transpose per batch), streamed operand = a
    constant [8, 136] augmented points_x matrix.
  - Remaining elementwise work is ~5 DVE ops + 2 ACT ops per 128-view
    tile, with per-view sums fused into the ops via accum_out.
  - Per-core output: num[128, 40], den[128, 40]; host does the final
    (num/den) reduction and the /V^2 (tiny: 40K values).
"""

import os
import sys

import numpy as np

for _p in ("/opt/trn_rl_repo",):
    if _p not in sys.path and os.path.isdir(_p):
        sys.path.insert(0, _p)

import concourse.bass as bass
import concourse.bacc as bacc
import concourse.tile as tile
from concourse import mybir
from concourse.bass_utils import run_bass_kernel_spmd
from concourse.masks import make_identity
from contextlib import ExitStack

F32 = mybir.dt.float32
NPTS = 68
ROWW = 15
VROW = NPTS * ROWW  # 1020 floats per view block
N_CORES = 8
V_TOTAL = 40000
V_CORE = V_TOTAL // N_CORES  # 5000
VPT = 128  # views per tile (partition dim)


def build_nc(v_core=V_CORE, tiles_per_chunk=10):
    """Build the single-core Bass program (same program runs SPMD on 8 cores)."""
    nt = (v_core + VPT - 1) // VPT
    B = min(tiles_per_chunk, nt)
    # ramp-up chunk schedule: small first chunks so compute starts early,
    # then full-size chunks at the DMA roofline
    chunk_sizes = []
    t = 0
    for sz in (max(1, B // 5), max(1, (2 * B) // 5)):
        if t + sz <= nt:
            chunk_sizes.append(sz)
            t += sz
    while t < nt:
        sz = min(B, nt - t)
        chunk_sizes.append(sz)
        t += sz

    nc = bacc.Bacc()
    y = nc.dram_tensor("y", [v_core * NPTS, ROWW], F32, kind="ExternalInput")
    xaug_d = nc.dram_tensor("xaug", [8, 2 * NPTS], F32, kind="ExternalInput")
    num_o = nc.dram_tensor("num", [VPT, nt], F32, kind="ExternalOutput")
    den_o = nc.dram_tensor("den", [VPT, nt], F32, kind="ExternalOutput")

    # [v, (l c)] view of the input: one view block = 1020 contiguous floats
    y2 = y.rearrange("(v l) c -> v (l c)", l=NPTS)

    AF = mybir.ActivationFunctionType
    ALU = mybir.AluOpType

    with ExitStack() as ctx:
        tc = ctx.enter_context(tile.TileContext(nc))
        consts = ctx.enter_context(tc.tile_pool(name="consts", bufs=1))
        slabs = ctx.enter_context(tc.tile_pool(name="slabs", bufs=2))
        hdr = ctx.enter_context(tc.tile_pool(name="hdr", bufs=2))
        work = ctx.enter_context(tc.tile_pool(name="work", bufs=3))
        outp = ctx.enter_context(tc.tile_pool(name="outp", bufs=1))
        psum_p = ctx.enter_context(tc.tile_pool(name="psum_p", bufs=3, space="PSUM"))
        psum_t = ctx.enter_context(tc.tile_pool(name="psum_t", bufs=2, space="PSUM"))

        identity = consts.tile([128, 128], F32)
        make_identity(nc, identity)

        # Constant streamed matrix for the projection matmul.
        # out[p, e*68+l] = sum_k lhsT[k, p] * xaug[k, e*68+l]
        #   rows 0..2 : X[l, d] in cols 0:68      (e = 0)
        #   row  3    : -1      in cols 0:68
        #   rows 4..6 : X[l, d] in cols 68:136    (e = 1)
        #   row  7    : -1      in cols 68:136
        xaug_f = consts.tile([8, 2 * NPTS], F32, name="xaug_f")
        nc.sync.dma_start(out=xaug_f, in_=xaug_d[:, :])

        NUM = outp.tile([VPT, nt], F32)
        DEN = outp.tile([VPT, nt], F32)

        t0 = 0
        for bc in chunk_sizes:
            v0 = t0 * VPT
            n_views = min(v_core - v0, bc * VPT)
            nf = n_views // VPT  # full tiles
            rem = n_views - nf * VPT

            slab = slabs.tile([VPT, B, VROW], F32, tag="slab")
            if nf > 0:
                src = y2[v0 : v0 + nf * VPT].rearrange("(w p) f -> p w f", p=VPT)
                nc.sync.dma_start(out=slab[:, 0:nf, :], in_=src)
            if rem > 0:
                src = y2[v0 + nf * VPT : v0 + n_views]
                nc.sync.dma_start(out=slab[0:rem, nf, :], in_=src)

            # ---- batched 3x3 inverse header math (views on partitions,
            # tiles of this chunk along the free dim; inputs read straight
            # from the slab with stride-1020 APs) ----
            def rr(i, j):
                return slab[:, 0:bc, 3 + 3 * i + j]

            def tt_(o, a, b, op):
                nc.vector.tensor_tensor(o, a, b, op=op)

            hout = hdr.tile([VPT, bc * 8], F32, tag="hout")
            hv = hout.rearrange("p (w k) -> p w k", k=8)

            def cof(dst, a1, b1, a2, b2):
                # dst = a1*b1 - a2*b2
                u = hdr.tile([VPT, bc], F32, tag="cof_u")
                v = hdr.tile([VPT, bc], F32, tag="cof_v")
                tt_(u, a1, b1, ALU.mult)
                tt_(v, a2, b2, ALU.mult)
                tt_(dst, u, v, ALU.subtract)
                return dst

            a00 = cof(hdr.tile([VPT, bc], F32, name="a00", tag="a00"), rr(1, 1), rr(2, 2), rr(1, 2), rr(2, 1))
            a10 = cof(hdr.tile([VPT, bc], F32, name="a10", tag="a10"), rr(1, 2), rr(2, 0), rr(1, 0), rr(2, 2))
            a20 = cof(hdr.tile([VPT, bc], F32, name="a20", tag="a20"), rr(1, 0), rr(2, 1), rr(1, 1), rr(2, 0))
            a01 = cof(hdr.tile([VPT, bc], F32, name="a01", tag="a01"), rr(0, 2), rr(2, 1), rr(0, 1), rr(2, 2))
            a11 = cof(hdr.tile([VPT, bc], F32, name="a11", tag="a11"), rr(0, 0), rr(2, 2), rr(0, 2), rr(2, 0))
            a21 = cof(hdr.tile([VPT, bc], F32, name="a21", tag="a21"), rr(0, 1), rr(2, 0), rr(0, 0), rr(2, 1))

            # det = r00*a00 + r01*a10 + r02*a20 ; rinv = 1/(det*scale)
            d1 = hdr.tile([VPT, bc], F32, tag="d1")
            d2 = hdr.tile([VPT, bc], F32, tag="d2")
            det = hdr.tile([VPT, bc], F32, tag="det")
            tt_(d1, rr(0, 0), a00, ALU.mult)
            tt_(d2, rr(0, 1), a10, ALU.mult)
            tt_(d1, d1, d2, ALU.add)
            tt_(d2, rr(0, 2), a20, ALU.mult)
            tt_(det, d1, d2, ALU.add)
            tt_(d1, det, slab[:, 0:bc, 2], ALU.mult)  # det * scale
            rinv = hdr.tile([VPT, bc], F32, tag="rinv")
            nc.vector.reciprocal(rinv, d1)

            # M columns 0..2 (k=0,1,2 / 4,5,6) and bias rows c~ (k=3,7)
            for k, adj in ((0, a00), (1, a10), (2, a20), (4, a01), (5, a11), (6, a21)):
                tt_(hv[:, :, k], adj, rinv, ALU.mult)
            for ke, k0 in ((3, 0), (7, 4)):
                u1 = hdr.tile([VPT, bc], F32, tag="u1")
                u2 = hdr.tile([VPT, bc], F32, tag="u2")
                tt_(u1, slab[:, 0:bc, 12], hv[:, :, k0 + 0], ALU.mult)
                tt_(u2, slab[:, 0:bc, 13], hv[:, :, k0 + 1], ALU.mult)
                tt_(u1, u1, u2, ALU.add)
                tt_(u2, slab[:, 0:bc, 14], hv[:, :, k0 + 2], ALU.mult)
                tt_(hv[:, :, ke], u1, u2, ALU.add)

            # ---- per-tile main compute ----
            for wi in range(bc):
                w = t0 + wi
                pv = slab[:, wi].rearrange("p (l c) -> p c l", c=ROWW)
                pt2 = pv[:, 0:2, :]  # [128, 2, 68] strided view of pt_y

                # transpose this tile's 8 weight rows to [8, 128]; engines
                # need partition bases 0/32/64/96, so per-tile tiles it is
                tps8 = psum_t.tile([8, 128], F32, tag="tps8", bufs=3)
                nc.tensor.transpose(tps8, hv[:, wi, :], identity)
                lhsT = work.tile([8, 128], F32, tag="lhsT", bufs=4)
                nc.scalar.copy(lhsT, tps8)

                proj = psum_p.tile([VPT, 2, NPTS], F32, tag="proj")
                nc.tensor.matmul(
                    proj.rearrange("p e l -> p (e l)"),
                    lhsT,
                    xaug_f,
                    start=True,
                    stop=True,
                )

                d_sb = work.tile([VPT, 2, NPTS], F32, tag="d_sb")
                nc.vector.tensor_tensor(d_sb, pt2, proj, op=ALU.subtract)
                sq = work.tile([VPT, 2, NPTS], F32, tag="sq")
                nc.scalar.activation(sq, d_sb, AF.Square)
                ss = work.tile([VPT, NPTS], F32, tag="ss")
                nc.vector.tensor_tensor(ss, sq[:, 0, :], sq[:, 1, :], op=ALU.add)
                m = work.tile([VPT, NPTS], F32, tag="m")
                nc.vector.tensor_tensor(m, pv[:, 0, :], pv[:, 1, :], op=ALU.max)
                mge = work.tile([VPT, NPTS], F32, tag="mge")
                nc.vector.tensor_scalar(
                    mge, m, 0.0, None, op0=ALU.is_ge, op1=ALU.add,
                    accum_out=DEN[:, w : w + 1],
                )
                msq = work.tile([VPT, NPTS], F32, tag="msq")
                nc.vector.tensor_tensor(msq, ss, mge, op=ALU.mult)
                dist = work.tile([VPT, NPTS], F32, tag="dist")
                nc.scalar.activation(
                    dist, msq, AF.Sqrt, accum_out=NUM[:, w : w + 1]
                )
            t0 += bc

        nc.sync.dma_start(out=num_o[:, :], in_=NUM)
        nc.sync.dma_start(out=den_o[:, :], in_=DEN)

    nc.compile()
    return nc, nt


_CACHE = {}


def _get_nc(v_core=V_CORE):
    key = v_core
    if key not in _CACHE:
        _CACHE[key] = build_nc(v_core)
    return _CACHE[key]


def make_xaug(points_x):
    """Host-built [8, 136] streamed constant for the projection matmul."""
    xa = np.zeros((8, 2 * NPTS), dtype=np.float32)
    xa[0:3, 0:NPTS] = points_x.T
    xa[3, 0:NPTS] = -1.0
    xa[4:7, NPTS:] = points_x.T
    xa[7, NPTS:] = -1.0
    return xa


def host_finish(nums, dens, v_core, v_total):
    """Combine per-core [128, nt] num/den partials into the scalar loss."""
    total = 0.0
    for num, den in zip(nums, dens):
        nt = num.shape[1]
        lv = num.astype(np.float64) / den.astype(np.float64)
        for w in range(nt):
            valid = min(VPT, v_core - w * VPT)
            total += lv[:valid, w].sum()
    return np.float32(total / (float(v_total) * float(v_total)))


def kernel(points_x, points_y):
    points_x = np.asarray(points_x, dtype=np.float32)
    points_y = np.asarray(points_y, dtype=np.float32)
    v_total = (points_y.shape[0] - NPTS) // NPTS
    v_core = v_total // N_CORES
    nc, nt = _get_nc(v_core)

    body = points_y[NPTS:]
    xa = make_xaug(points_x)
    in_maps = []
    for c in range(N_CORES):
        shard = np.ascontiguousarray(
            body[c * v_core * NPTS : (c + 1) * v_core * NPTS]
        )
        in_maps.append({"y": shard, "xaug": xa})

    res = run_bass_kernel_spmd(nc, in_maps, list(range(N_CORES)))
    nums = [res.results[c]["num"] for c in range(N_CORES)]
    dens = [res.results[c]["den"] for c in range(N_CORES)]
    return host_finish(nums, dens, v_core, v_total)



# revision 3
# speedup vs baseline: 1.1282x; 1.1282x over previous
"""Trainium2 Bass kernel for nn_Loss2D_57432302682561 (v2).

Math per view v (V = 40000 views, 68 landmarks each):
    y block  = points_y[68 + 68v : 68 + 68(v+1)]          # [68, 15]
    pt       = y[:, 0:2]                                   # target 2D points
    scale    = y[0, 2];  R = y[0, 3:12].reshape(3,3);  t = y[0, 12:15]
    M        = inv(scale * R) = adj(R) / (scale * det(R))  # [3, 3]
    proj     = (points_x - t) @ M  -> first 2 cols         # [68, 2]
    mask     = (pt[:,0] >= 0) | (pt[:,1] >= 0)
    dist     = sqrt(sum((pt - proj)^2, -1))
    loss_v   = sum(dist * mask) / sum(mask)
    out      = sum_v loss_v / V^2

v2 design (8 NeuronCores, data-parallel over views; DMA-roofline bound):
  - Chunks of 12 tiles (128 views each) double-buffered; one contiguous
    slab DMA per chunk (full-bandwidth 4080B descriptors).
  - Header math (3x3 inverse) via duplicated-row cross products:
    inv(R) cols 0/1 = cross(r1,r2), cross(r2,r0) over scale*det.  ~20 wide
    ops per chunk instead of 41 tiny ops, split between GpSimd and DVE.
  - Weights for ALL tiles of a chunk transposed in ONE PE transpose
    ([128 views, 128 weight-cols] -> PSUM), one PSUM->SBUF copy.  Weight
    cols are grouped 32 per 3-tile group (24 data + 8 pad) so matmul
    lhsT slices land on legal partition bases {0,32,64,96}.
  - Projection: one PE matmul per 3-tile group with a block-diagonal
    [24, 408] streamed matrix (replicated in all 4 SBUF quadrants so
    lhsT/rhs partition bases match).
  - Elementwise work grouped over 3 tiles per op (DVE), mask max on
    GpSimd, Square/Sqrt on ACT, per-view mask count via fused
    tensor_scalar accum (cheap DVE accumulator), per-view distance sum
    via DVE pool_avg windows.
  - Per-core output: num[128, nt] (mean masked dist), den[128, nt]
    (mask count); host computes 68*num/den and the final reduction.
"""

import os
import sys

import numpy as np

for _p in ("/opt/trn_rl_repo",):
    if _p not in sys.path and os.path.isdir(_p):
        sys.path.insert(0, _p)

import concourse.bass as bass
import concourse.bacc as bacc
import concourse.tile as tile
from concourse import mybir
from concourse.bass_utils import run_bass_kernel_spmd
from concourse.masks import make_identity
from contextlib import ExitStack

F32 = mybir.dt.float32
NPTS = 68
ROWW = 15
VROW = NPTS * ROWW  # 1020 floats per view block
N_CORES = 8
V_TOTAL = 40000
V_CORE = V_TOTAL // N_CORES  # 5000
VPT = 128  # views per tile (partition dim)
B = 12     # tiles per chunk
GR = 3     # tiles per matmul group


def build_nc(v_core=V_CORE):
    nt = (v_core + VPT - 1) // VPT
    chunk_sizes = []
    t = 0
    while t < nt:
        sz = min(B, nt - t)
        chunk_sizes.append(sz)
        t += sz

    nc = bacc.Bacc()
    y = nc.dram_tensor("y", [v_core * NPTS, ROWW], F32, kind="ExternalInput")
    xq_d = nc.dram_tensor("xaug3", [128, GR * 2 * NPTS], F32, kind="ExternalInput")
    num_o = nc.dram_tensor("num", [VPT, nt], F32, kind="ExternalOutput")
    den_o = nc.dram_tensor("den", [VPT, nt], F32, kind="ExternalOutput")

    # [v, (l c)] view of the input: one view block = 1020 contiguous floats
    y2 = y.rearrange("(v l) c -> v (l c)", l=NPTS)

    AF = mybir.ActivationFunctionType
    ALU = mybir.AluOpType

    with ExitStack() as ctx:
        tc = ctx.enter_context(tile.TileContext(nc))
        consts = ctx.enter_context(tc.tile_pool(name="consts", bufs=1))
        slabs = ctx.enter_context(tc.tile_pool(name="slabs", bufs=2))
        hdr = ctx.enter_context(tc.tile_pool(name="hdr", bufs=2))
        hvp = ctx.enter_context(tc.tile_pool(name="hvp", bufs=2))
        lhsp = ctx.enter_context(tc.tile_pool(name="lhsp", bufs=2))
        work = ctx.enter_context(tc.tile_pool(name="work", bufs=3))
        outp = ctx.enter_context(tc.tile_pool(name="outp", bufs=1))
        psum_p = ctx.enter_context(tc.tile_pool(name="psum_p", bufs=4, space="PSUM"))
        psum_t = ctx.enter_context(tc.tile_pool(name="psum_t", bufs=2, space="PSUM"))

        identity = consts.tile([128, 128], F32)
        make_identity(nc, identity)

        # Streamed projection matrix: block-diagonal [24, 408] (three [8,136]
        # xaug blocks), replicated in all four partition quadrants so the
        # matmul rhs partition base can match any lhsT group base.
        xq = consts.tile([128, GR * 2 * NPTS], F32, name="xq")
        nc.sync.dma_start(out=xq, in_=xq_d[:, :])

        NUM = outp.tile([VPT, nt], F32)
        DEN = outp.tile([VPT, nt], F32)

        t0 = 0
        for bc in chunk_sizes:
            v0 = t0 * VPT
            n_views = min(v_core - v0, bc * VPT)
            nf = n_views // VPT  # full tiles
            rem = n_views - nf * VPT
            ng = (bc + GR - 1) // GR          # matmul groups this chunk
            ngf = bc // GR                    # full (3-tile) groups
            nf3 = ngf * GR                    # tiles covered by full groups
            gs_t = bc - nf3                   # tail group size (0..2)

            slab = slabs.tile([VPT, B, VROW], F32, tag="slab")
            if nf > 0:
                src = y2[v0 : v0 + nf * VPT].rearrange("(w p) f -> p w f", p=VPT)
                nc.sync.dma_start(out=slab[:, 0:nf, :], in_=src)
            if rem > 0:
                src = y2[v0 + nf * VPT : v0 + n_views]
                nc.sync.dma_start(out=slab[0:rem, nf, :], in_=src)
                # invalid partitions of the remainder tile: fill with real
                # (replicated) views so all lanes stay finite; host ignores
                # their num/den entries
                nc.sync.dma_start(
                    out=slab[rem:VPT, nf, :], in_=y2[v0 : v0 + VPT - rem]
                )

            # ---- header math: M = inv(scale*R) cols 0,1 + bias rows ----
            # hv[:, 32g + 8t + k], k in 0..7: [M00,M10,M20,c0,M01,M11,M21,c1]
            hv = hvp.tile([VPT, 128], F32, tag="hv")
            nc.gpsimd.memset(hv, 0.0)

            # duplicated row buffers for cross products:
            # A = [r1 r1-dup | r2 r2-dup], B = [r2 r2-dup | r0 r0-dup]
            A = hdr.tile([VPT, B, 2, 5], F32, tag="A")
            Bt = hdr.tile([VPT, B, 2, 5], F32, tag="Bt")
            r12 = slab[:, 0:bc, 6:12].rearrange("p w (r c) -> p w r c", r=2)
            nc.scalar.copy(A[:, 0:bc, :, 0:3], r12)
            nc.scalar.copy(A[:, 0:bc, :, 3:5], r12[:, :, :, 0:2])
            nc.scalar.copy(Bt[:, 0:bc, 0, 0:3], slab[:, 0:bc, 9:12])
            nc.scalar.copy(Bt[:, 0:bc, 0, 3:5], slab[:, 0:bc, 9:11])
            nc.scalar.copy(Bt[:, 0:bc, 1, 0:3], slab[:, 0:bc, 3:6])
            nc.scalar.copy(Bt[:, 0:bc, 1, 3:5], slab[:, 0:bc, 3:5])

            # X[:, w, 0, :] = cross(r1, r2);  X[:, w, 1, :] = cross(r2, r0)
            P = hdr.tile([VPT, B, 2, 3], F32, tag="P")
            Q = hdr.tile([VPT, B, 2, 3], F32, tag="Q")
            X = hdr.tile([VPT, B, 2, 3], F32, tag="X")
            nc.vector.tensor_tensor(
                P[:, 0:bc], A[:, 0:bc, :, 1:4], Bt[:, 0:bc, :, 2:5], op=ALU.mult
            )
            nc.vector.tensor_tensor(
                Q[:, 0:bc], A[:, 0:bc, :, 2:5], Bt[:, 0:bc, :, 1:4], op=ALU.mult
            )
            nc.vector.tensor_tensor(
                X[:, 0:bc], P[:, 0:bc], Q[:, 0:bc], op=ALU.subtract
            )

            # det = r0 . cross(r1, r2);  rinv = 1 / (scale * det)
            T3 = hdr.tile([VPT, B, 4], F32, tag="T3")
            nc.vector.tensor_tensor(
                T3[:, 0:bc, 0:3], X[:, 0:bc, 0, :], slab[:, 0:bc, 3:6], op=ALU.mult
            )
            da = hdr.tile([VPT, B], F32, tag="da")
            nc.vector.tensor_tensor(
                da[:, 0:bc], T3[:, 0:bc, 0], T3[:, 0:bc, 1], op=ALU.add
            )
            det = hdr.tile([VPT, B], F32, tag="det")
            nc.vector.tensor_tensor(
                det[:, 0:bc], da[:, 0:bc], T3[:, 0:bc, 2], op=ALU.add
            )
            u = hdr.tile([VPT, B], F32, tag="u")
            nc.vector.tensor_tensor(
                u[:, 0:bc], det[:, 0:bc], slab[:, 0:bc, 2], op=ALU.mult
            )
            rinv = hdr.tile([VPT, B], F32, tag="rinv")
            nc.vector.reciprocal(rinv[:, 0:bc], u[:, 0:bc])

            # M columns into hv (k 0..2 e=0, k 4..6 e=1): X * rinv
            hvk = hv.rearrange("p (g t k) -> p g t k", t=4, k=8)  # [128,4,4,8]
            hv5 = hvk[:, :, 0:3, :].rearrange("p g t (e c) -> p g t e c", e=2)
            W6 = hdr.tile([VPT, B, 2, 4], F32, tag="W6")
            if ngf > 0:
                hvM = hv5[:, 0:ngf, :, :, 0:3]
                X5 = X[:, 0:nf3].rearrange("p (g t) e c -> p g t e c", t=GR)
                r5 = (
                    rinv[:, 0:nf3]
                    .rearrange("p (g t) -> p g t", t=GR)
                    .unsqueeze(3).unsqueeze(4)
                    .broadcast_to([VPT, ngf, GR, 2, 3])
                )
                nc.vector.tensor_tensor(hvM, X5, r5, op=ALU.mult)
                # bias pre-sums: W6 = Mcol * t  (summed to c_e below)
                t5 = (
                    slab[:, 0:nf3, 12:15]
                    .rearrange("p (g t) c -> p g t c", t=GR)
                    .unsqueeze(3)
                    .broadcast_to([VPT, ngf, GR, 2, 3])
                )
                nc.vector.tensor_tensor(
                    W6[:, 0:nf3, :, 0:3].rearrange("p (g t) e c -> p g t e c", t=GR),
                    hvM, t5, op=ALU.mult,
                )
            if gs_t > 0:
                hvMt = hv5[:, ngf, 0:gs_t, :, 0:3]
                rt = (
                    rinv[:, nf3:bc].unsqueeze(2).unsqueeze(3)
                    .broadcast_to([VPT, gs_t, 2, 3])
                )
                nc.vector.tensor_tensor(hvMt, X[:, nf3:bc], rt, op=ALU.mult)
                tt5 = (
                    slab[:, nf3:bc, 12:15].unsqueeze(2)
                    .broadcast_to([VPT, gs_t, 2, 3])
                )
                nc.vector.tensor_tensor(W6[:, nf3:bc, :, 0:3], hvMt, tt5, op=ALU.mult)

            # bias c_e = sum_c W6[..., c], written straight into hv k=3,7
            cb = hdr.tile([VPT, B, 2], F32, tag="cb")
            nc.vector.tensor_tensor(
                cb[:, 0:bc], W6[:, 0:bc, :, 0], W6[:, 0:bc, :, 1], op=ALU.add
            )
            hvB5 = hvk[:, :, 0:3, :].rearrange("p g t (e c) -> p g t e c", e=2)
            if ngf > 0:
                nc.vector.tensor_tensor(
                    hvB5[:, 0:ngf, :, :, 3:4],
                    cb[:, 0:nf3]
                    .rearrange("p (g t) e -> p g t e", t=GR)
                    .unsqueeze(4),
                    W6[:, 0:nf3, :, 2:3]
                    .rearrange("p (g t) e c -> p g t e c", t=GR),
                    op=ALU.add,
                )
            if gs_t > 0:
                nc.vector.tensor_tensor(
                    hvB5[:, ngf, 0:gs_t, :, 3:4],
                    cb[:, nf3:bc].unsqueeze(3),
                    W6[:, nf3:bc, :, 2:3],
                    op=ALU.add,
                )

            # ---- transpose all weights for this chunk in one shot ----
            F = 32 * ng
            tps = psum_t.tile([128, 128], F32, tag="tps")
            nc.tensor.transpose(tps[0:F, :], hv[:, 0:F], identity)
            lhsT = lhsp.tile([128, 128], F32, tag="lhsT")
            nc.scalar.copy(lhsT[0:F, :], tps[0:F, :])

            # ---- per-group main compute ----
            for g in range(ng):
                lw0 = g * GR
                gs = min(GR, bc - lw0)
                w0 = t0 + lw0
                K = 8 * gs
                rb = 32 * g

                proj = psum_p.tile([VPT, GR, 2, NPTS], F32, tag="proj")
                nc.tensor.matmul(
                    proj[:, 0:gs],
                    lhsT[rb : rb + K, :],
                    xq[rb : rb + K, 0 : gs * 2 * NPTS],
                    start=True,
                    stop=True,
                    tile_position=(rb, 0),
                )

                ptv = slab[:, lw0 : lw0 + gs, :].rearrange(
                    "p j (l c) -> p j c l", c=ROWW
                )
                d = work.tile([VPT, GR, 2, NPTS], F32, tag="d")
                nc.vector.tensor_tensor(
                    d[:, 0:gs], ptv[:, :, 0:2, :], proj[:, 0:gs], op=ALU.subtract
                )
                sq = work.tile([VPT, GR, 2, NPTS], F32, tag="sq")
                nc.scalar.activation(sq[:, 0:gs], d[:, 0:gs], AF.Square)
                ss = work.tile([VPT, GR, NPTS], F32, tag="ss")
                nc.vector.tensor_tensor(
                    ss[:, 0:gs], sq[:, 0:gs, 0, :], sq[:, 0:gs, 1, :], op=ALU.add
                )
                m = work.tile([VPT, GR, NPTS], F32, tag="m")
                nc.vector.tensor_tensor(
                    m[:, 0:gs], ptv[:, :, 0, :], ptv[:, :, 1, :], op=ALU.max
                )
                mge = work.tile([VPT, GR, NPTS], F32, tag="mge")
                for j in range(gs):
                    w = w0 + j
                    nc.vector.tensor_scalar(
                        mge[:, j], m[:, j], 0.0, None, op0=ALU.is_ge, op1=ALU.add,
                        accum_out=DEN[:, w : w + 1],
                    )
                msq = work.tile([VPT, GR, NPTS], F32, tag="msq")
                nc.vector.tensor_tensor(
                    msq[:, 0:gs], ss[:, 0:gs], mge[:, 0:gs], op=ALU.mult
                )
                dist = work.tile([VPT, GR, NPTS], F32, tag="dist")
                for j in range(gs):
                    w = w0 + j
                    nc.scalar.activation(
                        dist[:, j], msq[:, j], AF.Sqrt,
                        accum_out=NUM[:, w : w + 1],
                    )
            t0 += bc

        nc.sync.dma_start(out=num_o[:, :], in_=NUM)
        nc.sync.dma_start(out=den_o[:, :], in_=DEN)

    nc.compile()
    return nc, nt


_CACHE = {}


def _get_nc(v_core=V_CORE):
    key = v_core
    if key not in _CACHE:
        _CACHE[key] = build_nc(v_core)
    return _CACHE[key]


def make_xaug3(points_x):
    """Host-built [128, 408] streamed constant: block-diag [24, 408] of
    three [8, 136] xaug blocks, replicated in all 4 partition quadrants."""
    xa = np.zeros((8, 2 * NPTS), dtype=np.float32)
    xa[0:3, 0:NPTS] = points_x.T
    xa[3, 0:NPTS] = -1.0
    xa[4:7, NPTS:] = points_x.T
    xa[7, NPTS:] = -1.0
    xq = np.zeros((128, GR * 2 * NPTS), dtype=np.float32)
    for q in range(4):
        for b in range(GR):
            xq[32 * q + 8 * b : 32 * q + 8 * b + 8,
               2 * NPTS * b : 2 * NPTS * (b + 1)] = xa
    return xq


def host_finish(nums, dens, v_core, v_total):
    """Combine per-core [128, nt] num/den partials into the scalar loss."""
    total = 0.0
    for num, den in zip(nums, dens):
        nt = num.shape[1]
        lv = num.astype(np.float64) / den.astype(np.float64)
        for w in range(nt):
            valid = min(VPT, v_core - w * VPT)
            total += lv[:valid, w].sum()
    return np.float32(total / (float(v_total) * float(v_total)))


def build_in_maps(points_x, points_y, v_core):
    body = points_y[NPTS:]
    xq = make_xaug3(points_x)
    in_maps = []
    for c in range(N_CORES):
        shard = np.ascontiguousarray(
            body[c * v_core * NPTS : (c + 1) * v_core * NPTS]
        )
        in_maps.append({"y": shard, "xaug3": xq})
    return in_maps


def kernel(points_x, points_y):
    points_x = np.asarray(points_x, dtype=np.float32)
    points_y = np.asarray(points_y, dtype=np.float32)
    v_total = (points_y.shape[0] - NPTS) // NPTS
    v_core = v_total // N_CORES
    nc, nt = _get_nc(v_core)

    in_maps = build_in_maps(points_x, points_y, v_core)
    res = run_bass_kernel_spmd(nc, in_maps, list(range(N_CORES)))
    nums = [res.results[c]["num"] for c in range(N_CORES)]
    dens = [res.results[c]["den"] for c in range(N_CORES)]
    return host_finish(nums, dens, v_core, v_total)


# revision 4
# speedup vs baseline: 1.2917x; 1.1449x over previous
"""Trainium2 Bass kernel for nn_Loss2D_57432302682561 (v2).

Math per view v (V = 40000 views, 68 landmarks each):
    y block  = points_y[68 + 68v : 68 + 68(v+1)]          # [68, 15]
    pt       = y[:, 0:2]                                   # target 2D points
    scale    = y[0, 2];  R = y[0, 3:12].reshape(3,3);  t = y[0, 12:15]
    M        = inv(scale * R) = adj(R) / (scale * det(R))  # [3, 3]
    proj     = (points_x - t) @ M  -> first 2 cols         # [68, 2]
    mask     = (pt[:,0] >= 0) | (pt[:,1] >= 0)
    dist     = sqrt(sum((pt - proj)^2, -1))
    loss_v   = sum(dist * mask) / sum(mask)
    out      = sum_v loss_v / V^2

v2 design (8 NeuronCores, data-parallel over views; DMA-roofline bound):
  - Chunks of 12 tiles (128 views each) double-buffered; one contiguous
    slab DMA per chunk (full-bandwidth 4080B descriptors).
  - Header math (3x3 inverse) via duplicated-row cross products:
    inv(R) cols 0/1 = cross(r1,r2), cross(r2,r0) over scale*det.  ~20 wide
    ops per chunk instead of 41 tiny ops (dup-copies on ACT, math on DVE).
  - Weights for ALL tiles of a chunk transposed in ONE PE transpose
    ([128 views, 128 weight-cols] -> PSUM), one PSUM->SBUF copy.  Weight
    cols are grouped 32 per 3-tile group (24 data + 8 pad) so matmul
    lhsT slices land on legal partition bases {0,32,64,96}.
  - Projection: one PE matmul per 3-tile group with a block-diagonal
    [24, 408] streamed matrix (replicated in all 4 SBUF quadrants so
    lhsT/rhs partition bases match).
  - Elementwise work grouped over 3 tiles per op (DVE); Square on ACT;
    per-view mask count via fused tensor_scalar accum (cheap DVE
    accumulator); per-view distance sum via per-tile ACT Sqrt accum.
  - Per-core output: num[128, nt] (masked dist sum), den[128, nt]
    (mask count); host computes num/den and the final reduction.
"""

import os
import sys

import numpy as np

for _p in ("/opt/trn_rl_repo",):
    if _p not in sys.path and os.path.isdir(_p):
        sys.path.insert(0, _p)

import concourse.bass as bass
import concourse.bacc as bacc
import concourse.tile as tile
from concourse import mybir
from concourse.bass_utils import run_bass_kernel_spmd
from concourse.masks import make_identity
from contextlib import ExitStack

F32 = mybir.dt.float32
NPTS = 68
ROWW = 15
VROW = NPTS * ROWW  # 1020 floats per view block
N_CORES = 8
V_TOTAL = 40000
V_CORE = V_TOTAL // N_CORES  # 5000
VPT = 128  # views per tile (partition dim)
B = 12     # tiles per chunk
GR = 3     # tiles per matmul group


def build_nc(v_core=V_CORE):
    nt = (v_core + VPT - 1) // VPT
    chunk_sizes = []
    t = 0
    while t < nt:
        sz = min(B, nt - t)
        chunk_sizes.append(sz)
        t += sz

    nc = bacc.Bacc()
    y = nc.dram_tensor("y", [v_core * NPTS, ROWW], F32, kind="ExternalInput")
    xq_d = nc.dram_tensor("xaug3", [128, GR * 2 * NPTS], F32, kind="ExternalInput")
    num_o = nc.dram_tensor("num", [VPT, nt], F32, kind="ExternalOutput")
    den_o = nc.dram_tensor("den", [VPT, nt], F32, kind="ExternalOutput")

    # [v, (l c)] view of the input: one view block = 1020 contiguous floats
    y2 = y.rearrange("(v l) c -> v (l c)", l=NPTS)

    AF = mybir.ActivationFunctionType
    ALU = mybir.AluOpType

    with ExitStack() as ctx:
        tc = ctx.enter_context(tile.TileContext(nc))
        consts = ctx.enter_context(tc.tile_pool(name="consts", bufs=1))
        slabs = ctx.enter_context(tc.tile_pool(name="slabs", bufs=2))
        hdr = ctx.enter_context(tc.tile_pool(name="hdr", bufs=2))
        hvp = ctx.enter_context(tc.tile_pool(name="hvp", bufs=2))
        lhsp = ctx.enter_context(tc.tile_pool(name="lhsp", bufs=2))
        work = ctx.enter_context(tc.tile_pool(name="work", bufs=3))
        outp = ctx.enter_context(tc.tile_pool(name="outp", bufs=1))
        psum_p = ctx.enter_context(tc.tile_pool(name="psum_p", bufs=4, space="PSUM"))
        psum_t = ctx.enter_context(tc.tile_pool(name="psum_t", bufs=2, space="PSUM"))

        identity = consts.tile([128, 128], F32)
        make_identity(nc, identity)

        # Streamed projection matrix: block-diagonal [24, 408] (three [8,136]
        # xaug blocks), replicated in all four partition quadrants so the
        # matmul rhs partition base can match any lhsT group base.
        xq = consts.tile([128, GR * 2 * NPTS], F32, name="xq")
        nc.sync.dma_start(out=xq, in_=xq_d[:, :])

        NUM = outp.tile([VPT, nt], F32)
        DEN = outp.tile([VPT, nt], F32)

        t0 = 0
        for bc in chunk_sizes:
            v0 = t0 * VPT
            n_views = min(v_core - v0, bc * VPT)
            nf = n_views // VPT  # full tiles
            rem = n_views - nf * VPT
            ng = (bc + GR - 1) // GR          # matmul groups this chunk
            ngf = bc // GR                    # full (3-tile) groups
            nf3 = ngf * GR                    # tiles covered by full groups
            gs_t = bc - nf3                   # tail group size (0..2)

            # Small early header DMA: 13 floats per view (scale, R, t).
            # Completes quickly, so header math + weight transpose for this
            # chunk overlap the previous chunk's bulk stream.
            hdrH = hdr.tile([VPT, B, 13], F32, tag="hdrH")
            if nf > 0:
                hsrc = y2[v0 : v0 + nf * VPT].rearrange(
                    "(w p) f -> p w f", p=VPT
                )[:, :, 2:15]
                nc.sync.dma_start(out=hdrH[:, 0:nf, :], in_=hsrc)
            if rem > 0:
                nc.sync.dma_start(
                    out=hdrH[0:rem, nf, :],
                    in_=y2[v0 + nf * VPT : v0 + n_views, 2:15],
                )
                # invalid partitions: real (replicated) headers keep all
                # lanes finite; host ignores their num/den entries
                nc.sync.dma_start(
                    out=hdrH[rem:VPT, nf, :], in_=y2[v0 : v0 + VPT - rem, 2:15]
                )

            # Bulk stream, one DMA per 3-tile group so elementwise compute
            # starts as soon as each group lands.
            slab = slabs.tile([VPT, B, VROW], F32, tag="slab")
            for g in range(ng):
                lw0 = g * GR
                gv0 = v0 + lw0 * VPT
                gnv = min(n_views - lw0 * VPT, GR * VPT)
                gnf = gnv // VPT
                grem = gnv - gnf * VPT
                if gnf > 0:
                    src = y2[gv0 : gv0 + gnf * VPT].rearrange(
                        "(w p) f -> p w f", p=VPT
                    )
                    nc.sync.dma_start(
                        out=slab[:, lw0 : lw0 + gnf, :], in_=src
                    )
                if grem > 0:
                    nc.sync.dma_start(
                        out=slab[0:grem, lw0 + gnf, :],
                        in_=y2[gv0 + gnf * VPT : gv0 + gnv],
                    )
                    nc.sync.dma_start(
                        out=slab[grem:VPT, lw0 + gnf, :],
                        in_=y2[v0 : v0 + VPT - grem],
                    )

            # ---- header math: M = inv(scale*R) cols 0,1 + bias rows ----
            # hv[:, 32g + 8t + k], k in 0..7: [M00,M10,M20,c0,M01,M11,M21,c1]
            hv = hvp.tile([VPT, 128], F32, tag="hv")
            nc.gpsimd.memset(hv, 0.0)

            # duplicated row buffers for cross products:
            # A = [r1 r1-dup | r2 r2-dup], B = [r2 r2-dup | r0 r0-dup]
            A = hdr.tile([VPT, B, 2, 5], F32, tag="A")
            Bt = hdr.tile([VPT, B, 2, 5], F32, tag="Bt")
            r12 = hdrH[:, 0:bc, 4:10].rearrange("p w (r c) -> p w r c", r=2)
            nc.scalar.copy(A[:, 0:bc, :, 0:3], r12)
            nc.scalar.copy(A[:, 0:bc, :, 3:5], r12[:, :, :, 0:2])
            nc.scalar.copy(Bt[:, 0:bc, 0, 0:3], hdrH[:, 0:bc, 7:10])
            nc.scalar.copy(Bt[:, 0:bc, 0, 3:5], hdrH[:, 0:bc, 7:9])
            nc.scalar.copy(Bt[:, 0:bc, 1, 0:3], hdrH[:, 0:bc, 1:4])
            nc.scalar.copy(Bt[:, 0:bc, 1, 3:5], hdrH[:, 0:bc, 1:3])

            # X[:, w, 0, :] = cross(r1, r2);  X[:, w, 1, :] = cross(r2, r0)
            P = hdr.tile([VPT, B, 2, 3], F32, tag="P")
            Q = hdr.tile([VPT, B, 2, 3], F32, tag="Q")
            X = hdr.tile([VPT, B, 2, 3], F32, tag="X")
            nc.vector.tensor_tensor(
                P[:, 0:bc], A[:, 0:bc, :, 1:4], Bt[:, 0:bc, :, 2:5], op=ALU.mult
            )
            nc.vector.tensor_tensor(
                Q[:, 0:bc], A[:, 0:bc, :, 2:5], Bt[:, 0:bc, :, 1:4], op=ALU.mult
            )
            nc.vector.tensor_tensor(
                X[:, 0:bc], P[:, 0:bc], Q[:, 0:bc], op=ALU.subtract
            )

            # det = r0 . cross(r1, r2);  rinv = 1 / (scale * det)
            T3 = hdr.tile([VPT, B, 4], F32, tag="T3")
            nc.vector.tensor_tensor(
                T3[:, 0:bc, 0:3], X[:, 0:bc, 0, :], hdrH[:, 0:bc, 1:4], op=ALU.mult
            )
            da = hdr.tile([VPT, B], F32, tag="da")
            nc.vector.tensor_tensor(
                da[:, 0:bc], T3[:, 0:bc, 0], T3[:, 0:bc, 1], op=ALU.add
            )
            det = hdr.tile([VPT, B], F32, tag="det")
            nc.vector.tensor_tensor(
                det[:, 0:bc], da[:, 0:bc], T3[:, 0:bc, 2], op=ALU.add
            )
            u = hdr.tile([VPT, B], F32, tag="u")
            nc.vector.tensor_tensor(
                u[:, 0:bc], det[:, 0:bc], hdrH[:, 0:bc, 0], op=ALU.mult
            )
            rinv = hdr.tile([VPT, B], F32, tag="rinv")
            nc.vector.reciprocal(rinv[:, 0:bc], u[:, 0:bc])

            # M columns into hv (k 0..2 e=0, k 4..6 e=1): X * rinv
            hvk = hv.rearrange("p (g t k) -> p g t k", t=4, k=8)  # [128,4,4,8]
            hv5 = hvk[:, :, 0:3, :].rearrange("p g t (e c) -> p g t e c", e=2)
            W6 = hdr.tile([VPT, B, 2, 4], F32, tag="W6")
            if ngf > 0:
                hvM = hv5[:, 0:ngf, :, :, 0:3]
                X5 = X[:, 0:nf3].rearrange("p (g t) e c -> p g t e c", t=GR)
                r5 = (
                    rinv[:, 0:nf3]
                    .rearrange("p (g t) -> p g t", t=GR)
                    .unsqueeze(3).unsqueeze(4)
                    .broadcast_to([VPT, ngf, GR, 2, 3])
                )
                nc.vector.tensor_tensor(hvM, X5, r5, op=ALU.mult)
                # bias pre-sums: W6 = Mcol * t  (summed to c_e below)
                t5 = (
                    hdrH[:, 0:nf3, 10:13]
                    .rearrange("p (g t) c -> p g t c", t=GR)
                    .unsqueeze(3)
                    .broadcast_to([VPT, ngf, GR, 2, 3])
                )
                nc.vector.tensor_tensor(
                    W6[:, 0:nf3, :, 0:3].rearrange("p (g t) e c -> p g t e c", t=GR),
                    hvM, t5, op=ALU.mult,
                )
            if gs_t > 0:
                hvMt = hv5[:, ngf, 0:gs_t, :, 0:3]
                rt = (
                    rinv[:, nf3:bc].unsqueeze(2).unsqueeze(3)
                    .broadcast_to([VPT, gs_t, 2, 3])
                )
                nc.vector.tensor_tensor(hvMt, X[:, nf3:bc], rt, op=ALU.mult)
                tt5 = (
                    hdrH[:, nf3:bc, 10:13].unsqueeze(2)
                    .broadcast_to([VPT, gs_t, 2, 3])
                )
                nc.vector.tensor_tensor(W6[:, nf3:bc, :, 0:3], hvMt, tt5, op=ALU.mult)

            # bias c_e = sum_c W6[..., c], written straight into hv k=3,7
            cb = hdr.tile([VPT, B, 2], F32, tag="cb")
            nc.vector.tensor_tensor(
                cb[:, 0:bc], W6[:, 0:bc, :, 0], W6[:, 0:bc, :, 1], op=ALU.add
            )
            hvB5 = hvk[:, :, 0:3, :].rearrange("p g t (e c) -> p g t e c", e=2)
            if ngf > 0:
                nc.vector.tensor_tensor(
                    hvB5[:, 0:ngf, :, :, 3:4],
                    cb[:, 0:nf3]
                    .rearrange("p (g t) e -> p g t e", t=GR)
                    .unsqueeze(4),
                    W6[:, 0:nf3, :, 2:3]
                    .rearrange("p (g t) e c -> p g t e c", t=GR),
                    op=ALU.add,
                )
            if gs_t > 0:
                nc.vector.tensor_tensor(
                    hvB5[:, ngf, 0:gs_t, :, 3:4],
                    cb[:, nf3:bc].unsqueeze(3),
                    W6[:, nf3:bc, :, 2:3],
                    op=ALU.add,
                )

            # ---- transpose all weights for this chunk in one shot ----
            F = 32 * ng
            tps = psum_t.tile([128, 128], F32, tag="tps")
            nc.tensor.transpose(tps[0:F, :], hv[:, 0:F], identity)
            lhsT = lhsp.tile([128, 128], F32, tag="lhsT")
            nc.scalar.copy(lhsT[0:F, :], tps[0:F, :])

            # ---- per-group main compute ----
            for g in range(ng):
                lw0 = g * GR
                gs = min(GR, bc - lw0)
                w0 = t0 + lw0
                K = 8 * gs
                rb = 32 * g

                proj = psum_p.tile([VPT, GR, 2, NPTS], F32, tag="proj")
                nc.tensor.matmul(
                    proj[:, 0:gs],
                    lhsT[rb : rb + K, :],
                    xq[rb : rb + K, 0 : gs * 2 * NPTS],
                    start=True,
                    stop=True,
                    tile_position=(rb, 0),
                )

                ptv = slab[:, lw0 : lw0 + gs, :].rearrange(
                    "p j (l c) -> p j c l", c=ROWW
                )
                d = work.tile([VPT, GR, 2, NPTS], F32, tag="d")
                nc.vector.tensor_tensor(
                    d[:, 0:gs], ptv[:, :, 0:2, :], proj[:, 0:gs], op=ALU.subtract
                )
                sq = work.tile([VPT, GR, 2, NPTS], F32, tag="sq")
                nc.scalar.activation(sq[:, 0:gs], d[:, 0:gs], AF.Square)
                ss = work.tile([VPT, GR, NPTS], F32, tag="ss")
                nc.vector.tensor_tensor(
                    ss[:, 0:gs], sq[:, 0:gs, 0, :], sq[:, 0:gs, 1, :], op=ALU.add
                )
                m = work.tile([VPT, GR, NPTS], F32, tag="m")
                nc.vector.tensor_tensor(
                    m[:, 0:gs], ptv[:, :, 0, :], ptv[:, :, 1, :], op=ALU.max
                )
                mge = work.tile([VPT, GR, NPTS], F32, tag="mge")
                for j in range(gs):
                    w = w0 + j
                    nc.vector.tensor_scalar(
                        mge[:, j], m[:, j], 0.0, None, op0=ALU.is_ge, op1=ALU.add,
                        accum_out=DEN[:, w : w + 1],
                    )
                msq = work.tile([VPT, GR, NPTS], F32, tag="msq")
                nc.vector.tensor_tensor(
                    msq[:, 0:gs], ss[:, 0:gs], mge[:, 0:gs], op=ALU.mult
                )
                dist = work.tile([VPT, GR, NPTS], F32, tag="dist")
                for j in range(gs):
                    w = w0 + j
                    nc.scalar.activation(
                        dist[:, j], msq[:, j], AF.Sqrt,
                        accum_out=NUM[:, w : w + 1],
                    )
            t0 += bc

        nc.sync.dma_start(out=num_o[:, :], in_=NUM)
        nc.sync.dma_start(out=den_o[:, :], in_=DEN)

    nc.compile()
    return nc, nt


_CACHE = {}


def _get_nc(v_core=V_CORE):
    key = v_core
    if key not in _CACHE:
        _CACHE[key] = build_nc(v_core)
    return _CACHE[key]


def make_xaug3(points_x):
    """Host-built [128, 408] streamed constant: block-diag [24, 408] of
    three [8, 136] xaug blocks, replicated in all 4 partition quadrants."""
    xa = np.zeros((8, 2 * NPTS), dtype=np.float32)
    xa[0:3, 0:NPTS] = points_x.T
    xa[3, 0:NPTS] = -1.0
    xa[4:7, NPTS:] = points_x.T
    xa[7, NPTS:] = -1.0
    xq = np.zeros((128, GR * 2 * NPTS), dtype=np.float32)
    for q in range(4):
        for b in range(GR):
            xq[32 * q + 8 * b : 32 * q + 8 * b + 8,
               2 * NPTS * b : 2 * NPTS * (b + 1)] = xa
    return xq


def host_finish(nums, dens, v_core, v_total):
    """Combine per-core [128, nt] num/den partials into the scalar loss."""
    total = 0.0
    for num, den in zip(nums, dens):
        nt = num.shape[1]
        lv = num.astype(np.float64) / den.astype(np.float64)
        for w in range(nt):
            valid = min(VPT, v_core - w * VPT)
            total += lv[:valid, w].sum()
    return np.float32(total / (float(v_total) * float(v_total)))


def build_in_maps(points_x, points_y, v_core):
    body = points_y[NPTS:]
    xq = make_xaug3(points_x)
    in_maps = []
    for c in range(N_CORES):
        shard = np.ascontiguousarray(
            body[c * v_core * NPTS : (c + 1) * v_core * NPTS]
        )
        in_maps.append({"y": shard, "xaug3": xq})
    return in_maps


def kernel(points_x, points_y):
    points_x = np.asarray(points_x, dtype=np.float32)
    points_y = np.asarray(points_y, dtype=np.float32)
    v_total = (points_y.shape[0] - NPTS) // NPTS
    v_core = v_total // N_CORES
    nc, nt = _get_nc(v_core)

    in_maps = build_in_maps(points_x, points_y, v_core)
    res = run_bass_kernel_spmd(nc, in_maps, list(range(N_CORES)))
    nums = [res.results[c]["num"] for c in range(N_CORES)]
    dens = [res.results[c]["den"] for c in range(N_CORES)]
    return host_finish(nums, dens, v_core, v_total)


# revision 5
# speedup vs baseline: 1.3258x; 1.0264x over previous
"""Trainium2 Bass kernel for nn_Loss2D_57432302682561 (v2).

Math per view v (V = 40000 views, 68 landmarks each):
    y block  = points_y[68 + 68v : 68 + 68(v+1)]          # [68, 15]
    pt       = y[:, 0:2]                                   # target 2D points
    scale    = y[0, 2];  R = y[0, 3:12].reshape(3,3);  t = y[0, 12:15]
    M        = inv(scale * R) = adj(R) / (scale * det(R))  # [3, 3]
    proj     = (points_x - t) @ M  -> first 2 cols         # [68, 2]
    mask     = (pt[:,0] >= 0) | (pt[:,1] >= 0)
    dist     = sqrt(sum((pt - proj)^2, -1))
    loss_v   = sum(dist * mask) / sum(mask)
    out      = sum_v loss_v / V^2

v2 design (8 NeuronCores, data-parallel over views; DMA-roofline bound):
  - Chunks of 12 tiles (128 views each) double-buffered; one contiguous
    slab DMA per chunk (full-bandwidth 4080B descriptors).
  - Header math (3x3 inverse) via duplicated-row cross products:
    inv(R) cols 0/1 = cross(r1,r2), cross(r2,r0) over scale*det.  ~20 wide
    ops per chunk instead of 41 tiny ops (dup-copies on ACT, math on DVE).
  - Weights for ALL tiles of a chunk transposed in ONE PE transpose
    ([128 views, 128 weight-cols] -> PSUM), one PSUM->SBUF copy.  Weight
    cols are grouped 32 per 3-tile group (24 data + 8 pad) so matmul
    lhsT slices land on legal partition bases {0,32,64,96}.
  - Projection: one PE matmul per 3-tile group with a block-diagonal
    [24, 408] streamed matrix (replicated in all 4 SBUF quadrants so
    lhsT/rhs partition bases match).
  - Elementwise work grouped over 3 tiles per op (DVE); Square on ACT;
    per-view mask count via fused tensor_scalar accum (cheap DVE
    accumulator); per-view distance sum via per-tile ACT Sqrt accum.
  - Per-core output: num[128, nt] (masked dist sum), den[128, nt]
    (mask count); host computes num/den and the final reduction.
"""

import os
import sys

import numpy as np

for _p in ("/opt/trn_rl_repo",):
    if _p not in sys.path and os.path.isdir(_p):
        sys.path.insert(0, _p)

import concourse.bass as bass
import concourse.bacc as bacc
import concourse.tile as tile
from concourse import mybir
from concourse.bass_utils import run_bass_kernel_spmd
from concourse.masks import make_identity
from contextlib import ExitStack

F32 = mybir.dt.float32
NPTS = 68
ROWW = 15
VROW = NPTS * ROWW  # 1020 floats per view block
N_CORES = 8
V_TOTAL = 40000
V_CORE = V_TOTAL // N_CORES  # 5000
VPT = 128  # views per tile (partition dim)
B = 12     # tiles per chunk
GR = 3     # tiles per matmul group


def build_nc(v_core=V_CORE):
    nt = (v_core + VPT - 1) // VPT
    chunk_sizes = []
    t = 0
    while t < nt:
        sz = min(B, nt - t)
        chunk_sizes.append(sz)
        t += sz

    nc = bacc.Bacc()
    y = nc.dram_tensor("y", [v_core * NPTS, ROWW], F32, kind="ExternalInput")
    xq_d = nc.dram_tensor("xaug3", [128, GR * 2 * NPTS], F32, kind="ExternalInput")
    num_o = nc.dram_tensor("num", [VPT, nt], F32, kind="ExternalOutput")
    den_o = nc.dram_tensor("den", [VPT, nt], F32, kind="ExternalOutput")

    # [v, (l c)] view of the input: one view block = 1020 contiguous floats
    y2 = y.rearrange("(v l) c -> v (l c)", l=NPTS)

    AF = mybir.ActivationFunctionType
    ALU = mybir.AluOpType

    with ExitStack() as ctx:
        tc = ctx.enter_context(tile.TileContext(nc))
        consts = ctx.enter_context(tc.tile_pool(name="consts", bufs=1))
        slabs = ctx.enter_context(tc.tile_pool(name="slabs", bufs=2))
        hdr = ctx.enter_context(tc.tile_pool(name="hdr", bufs=2))
        hvp = ctx.enter_context(tc.tile_pool(name="hvp", bufs=2))
        lhsp = ctx.enter_context(tc.tile_pool(name="lhsp", bufs=2))
        work = ctx.enter_context(tc.tile_pool(name="work", bufs=3))
        outp = ctx.enter_context(tc.tile_pool(name="outp", bufs=1))
        psum_p = ctx.enter_context(tc.tile_pool(name="psum_p", bufs=4, space="PSUM"))
        psum_t = ctx.enter_context(tc.tile_pool(name="psum_t", bufs=2, space="PSUM"))

        identity = consts.tile([128, 128], F32)
        make_identity(nc, identity)

        # Streamed projection matrix: block-diagonal [24, 408] (three [8,136]
        # xaug blocks), replicated in all four partition quadrants so the
        # matmul rhs partition base can match any lhsT group base.
        xq = consts.tile([128, GR * 2 * NPTS], F32, name="xq")
        nc.sync.dma_start(out=xq, in_=xq_d[:, :])

        NUM = outp.tile([VPT, nt], F32)
        DEN = outp.tile([VPT, nt], F32)

        t0 = 0
        for bc in chunk_sizes:
            v0 = t0 * VPT
            n_views = min(v_core - v0, bc * VPT)
            nf = n_views // VPT  # full tiles
            rem = n_views - nf * VPT
            ng = (bc + GR - 1) // GR          # matmul groups this chunk
            ngf = bc // GR                    # full (3-tile) groups
            nf3 = ngf * GR                    # tiles covered by full groups
            gs_t = bc - nf3                   # tail group size (0..2)

            # Small early header DMA: 13 floats per view (scale, R, t).
            # Completes quickly, so header math + weight transpose for this
            # chunk overlap the previous chunk's bulk stream.
            hdrH = hdr.tile([VPT, B, 13], F32, tag="hdrH")
            if nf > 0:
                hsrc = y2[v0 : v0 + nf * VPT].rearrange(
                    "(w p) f -> p w f", p=VPT
                )[:, :, 2:15]
                nc.sync.dma_start(out=hdrH[:, 0:nf, :], in_=hsrc)
            if rem > 0:
                nc.sync.dma_start(
                    out=hdrH[0:rem, nf, :],
                    in_=y2[v0 + nf * VPT : v0 + n_views, 2:15],
                )
                # invalid partitions: real (replicated) headers keep all
                # lanes finite; host ignores their num/den entries
                nc.sync.dma_start(
                    out=hdrH[rem:VPT, nf, :], in_=y2[v0 : v0 + VPT - rem, 2:15]
                )

            # Bulk stream, one DMA per 3-tile group so elementwise compute
            # starts as soon as each group lands.
            slab = slabs.tile([VPT, B, VROW], F32, tag="slab")
            for g in range(ng):
                lw0 = g * GR
                gv0 = v0 + lw0 * VPT
                gnv = min(n_views - lw0 * VPT, GR * VPT)
                gnf = gnv // VPT
                grem = gnv - gnf * VPT
                if gnf > 0:
                    src = y2[gv0 : gv0 + gnf * VPT].rearrange(
                        "(w p) f -> p w f", p=VPT
                    )
                    nc.sync.dma_start(
                        out=slab[:, lw0 : lw0 + gnf, :], in_=src
                    )
                if grem > 0:
                    nc.sync.dma_start(
                        out=slab[0:grem, lw0 + gnf, :],
                        in_=y2[gv0 + gnf * VPT : gv0 + gnv],
                    )
                    nc.sync.dma_start(
                        out=slab[grem:VPT, lw0 + gnf, :],
                        in_=y2[v0 : v0 + VPT - grem],
                    )

            # ---- header math: M = inv(scale*R) cols 0,1 + bias rows ----
            # hv[:, 32g + 8t + k], k in 0..7: [M00,M10,M20,c0,M01,M11,M21,c1]
            hv = hvp.tile([VPT, 128], F32, tag="hv")
            nc.gpsimd.memset(hv, 0.0)

            # duplicated row buffers for cross products:
            # A = [r1 r1-dup | r2 r2-dup], B = [r2 r2-dup | r0 r0-dup]
            A = hdr.tile([VPT, B, 2, 5], F32, tag="A")
            Bt = hdr.tile([VPT, B, 2, 5], F32, tag="Bt")
            r12 = hdrH[:, 0:bc, 4:10].rearrange("p w (r c) -> p w r c", r=2)
            nc.scalar.copy(A[:, 0:bc, :, 0:3], r12)
            nc.scalar.copy(A[:, 0:bc, :, 3:5], r12[:, :, :, 0:2])
            nc.scalar.copy(Bt[:, 0:bc, 0, 0:3], hdrH[:, 0:bc, 7:10])
            nc.scalar.copy(Bt[:, 0:bc, 0, 3:5], hdrH[:, 0:bc, 7:9])
            nc.scalar.copy(Bt[:, 0:bc, 1, 0:3], hdrH[:, 0:bc, 1:4])
            nc.scalar.copy(Bt[:, 0:bc, 1, 3:5], hdrH[:, 0:bc, 1:3])

            # X[:, w, 0, :] = cross(r1, r2);  X[:, w, 1, :] = cross(r2, r0)
            P = hdr.tile([VPT, B, 2, 3], F32, tag="P")
            Q = hdr.tile([VPT, B, 2, 3], F32, tag="Q")
            X = hdr.tile([VPT, B, 2, 3], F32, tag="X")
            nc.vector.tensor_tensor(
                P[:, 0:bc], A[:, 0:bc, :, 1:4], Bt[:, 0:bc, :, 2:5], op=ALU.mult
            )
            nc.vector.tensor_tensor(
                Q[:, 0:bc], A[:, 0:bc, :, 2:5], Bt[:, 0:bc, :, 1:4], op=ALU.mult
            )
            nc.vector.tensor_tensor(
                X[:, 0:bc], P[:, 0:bc], Q[:, 0:bc], op=ALU.subtract
            )

            # det = r0 . cross(r1, r2);  rinv = 1 / (scale * det)
            T3 = hdr.tile([VPT, B, 4], F32, tag="T3")
            nc.vector.tensor_tensor(
                T3[:, 0:bc, 0:3], X[:, 0:bc, 0, :], hdrH[:, 0:bc, 1:4], op=ALU.mult
            )
            da = hdr.tile([VPT, B], F32, tag="da")
            nc.vector.tensor_tensor(
                da[:, 0:bc], T3[:, 0:bc, 0], T3[:, 0:bc, 1], op=ALU.add
            )
            det = hdr.tile([VPT, B], F32, tag="det")
            nc.vector.tensor_tensor(
                det[:, 0:bc], da[:, 0:bc], T3[:, 0:bc, 2], op=ALU.add
            )
            # u = -(det * scale): the whole weight set (M cols and biases)
            # is negated so the projection matmul computes -proj and pt can
            # be ADDED into the same PSUM bank via a second matmul,
            # yielding d = pt - proj with no DVE subtract.
            u = hdr.tile([VPT, B], F32, tag="u")
            nc.vector.scalar_tensor_tensor(
                u[:, 0:bc], det[:, 0:bc], -1.0, hdrH[:, 0:bc, 0],
                op0=ALU.mult, op1=ALU.mult,
            )
            rinv = hdr.tile([VPT, B], F32, tag="rinv")
            nc.vector.reciprocal(rinv[:, 0:bc], u[:, 0:bc])

            # M columns into hv (k 0..2 e=0, k 4..6 e=1): X * rinv
            hvk = hv.rearrange("p (g t k) -> p g t k", t=4, k=8)  # [128,4,4,8]
            hv5 = hvk[:, :, 0:3, :].rearrange("p g t (e c) -> p g t e c", e=2)
            W6 = hdr.tile([VPT, B, 2, 4], F32, tag="W6")
            if ngf > 0:
                hvM = hv5[:, 0:ngf, :, :, 0:3]
                X5 = X[:, 0:nf3].rearrange("p (g t) e c -> p g t e c", t=GR)
                r5 = (
                    rinv[:, 0:nf3]
                    .rearrange("p (g t) -> p g t", t=GR)
                    .unsqueeze(3).unsqueeze(4)
                    .broadcast_to([VPT, ngf, GR, 2, 3])
                )
                nc.vector.tensor_tensor(hvM, X5, r5, op=ALU.mult)
                # bias pre-sums: W6 = Mcol * t  (summed to c_e below)
                t5 = (
                    hdrH[:, 0:nf3, 10:13]
                    .rearrange("p (g t) c -> p g t c", t=GR)
                    .unsqueeze(3)
                    .broadcast_to([VPT, ngf, GR, 2, 3])
                )
                nc.vector.tensor_tensor(
                    W6[:, 0:nf3, :, 0:3].rearrange("p (g t) e c -> p g t e c", t=GR),
                    hvM, t5, op=ALU.mult,
                )
            if gs_t > 0:
                hvMt = hv5[:, ngf, 0:gs_t, :, 0:3]
                rt = (
                    rinv[:, nf3:bc].unsqueeze(2).unsqueeze(3)
                    .broadcast_to([VPT, gs_t, 2, 3])
                )
                nc.vector.tensor_tensor(hvMt, X[:, nf3:bc], rt, op=ALU.mult)
                tt5 = (
                    hdrH[:, nf3:bc, 10:13].unsqueeze(2)
                    .broadcast_to([VPT, gs_t, 2, 3])
                )
                nc.vector.tensor_tensor(W6[:, nf3:bc, :, 0:3], hvMt, tt5, op=ALU.mult)

            # bias c_e = sum_c W6[..., c], written straight into hv k=3,7
            cb = hdr.tile([VPT, B, 2], F32, tag="cb")
            nc.vector.tensor_tensor(
                cb[:, 0:bc], W6[:, 0:bc, :, 0], W6[:, 0:bc, :, 1], op=ALU.add
            )
            hvB5 = hvk[:, :, 0:3, :].rearrange("p g t (e c) -> p g t e c", e=2)
            if ngf > 0:
                nc.vector.tensor_tensor(
                    hvB5[:, 0:ngf, :, :, 3:4],
                    cb[:, 0:nf3]
                    .rearrange("p (g t) e -> p g t e", t=GR)
                    .unsqueeze(4),
                    W6[:, 0:nf3, :, 2:3]
                    .rearrange("p (g t) e c -> p g t e c", t=GR),
                    op=ALU.add,
                )
            if gs_t > 0:
                nc.vector.tensor_tensor(
                    hvB5[:, ngf, 0:gs_t, :, 3:4],
                    cb[:, nf3:bc].unsqueeze(3),
                    W6[:, nf3:bc, :, 2:3],
                    op=ALU.add,
                )

            # ---- transpose all weights for this chunk in one shot ----
            F = 32 * ng
            tps = psum_t.tile([128, 128], F32, tag="tps")
            nc.tensor.transpose(tps[0:F, :], hv[:, 0:F], identity)
            lhsT = lhsp.tile([128, 128], F32, tag="lhsT")
            nc.scalar.copy(lhsT[0:F, :], tps[0:F, :])

            # ---- per-group main compute (software-pipelined) ----
            # stage per group g: PE injects pt and accumulates -proj in
            # PSUM (d = pt - proj), ACT squares it, DVE masks/sums, ACT
            # sqrts, DVE accumulates NUM one group later (so no engine
            # queue head ever waits on a just-issued producer).
            pend = None  # (dist, w0, gs) awaiting NUM accumulation
            for g in range(ng):
                lw0 = g * GR
                gs = min(GR, bc - lw0)
                w0 = t0 + lw0
                K = 8 * gs
                rb = 32 * g

                ptv = slab[:, lw0 : lw0 + gs, :].rearrange(
                    "p j (l c) -> p j c l", c=ROWW
                )
                proj = psum_p.tile([VPT, GR, 2, NPTS], F32, tag="proj")
                nc.tensor.matmul(
                    proj[:, 0:gs],
                    identity,
                    ptv[:, :, 0:2, :],
                    start=True,
                    stop=False,
                    tile_position=(0, 0),
                )
                nc.tensor.matmul(
                    proj[:, 0:gs],
                    lhsT[rb : rb + K, :],
                    xq[rb : rb + K, 0 : gs * 2 * NPTS],
                    start=False,
                    stop=True,
                    tile_position=(rb, 0),
                )

                sq = work.tile([VPT, GR, 2, NPTS], F32, tag="sq")
                nc.scalar.activation(sq[:, 0:gs], proj[:, 0:gs], AF.Square)

                # mask ops first: they depend only on the slab, keeping DVE
                # busy while ACT squares
                m = work.tile([VPT, GR, NPTS], F32, tag="m")
                nc.vector.tensor_tensor(
                    m[:, 0:gs], ptv[:, :, 0, :], ptv[:, :, 1, :], op=ALU.max
                )
                mge = work.tile([VPT, GR, NPTS], F32, tag="mge")
                for j in range(gs):
                    w = w0 + j
                    nc.vector.tensor_scalar(
                        mge[:, j], m[:, j], 0.0, None, op0=ALU.is_ge, op1=ALU.add,
                        accum_out=DEN[:, w : w + 1],
                    )
                ss = work.tile([VPT, GR, NPTS], F32, tag="ss")
                nc.vector.tensor_tensor(
                    ss[:, 0:gs], sq[:, 0:gs, 0, :], sq[:, 0:gs, 1, :], op=ALU.add
                )
                msq = work.tile([VPT, GR, NPTS], F32, tag="msq")
                nc.vector.tensor_tensor(
                    msq[:, 0:gs], ss[:, 0:gs], mge[:, 0:gs], op=ALU.mult
                )
                dist = work.tile([VPT, GR, NPTS], F32, tag="dist")
                nc.scalar.activation(dist[:, 0:gs], msq[:, 0:gs], AF.Sqrt)

                if pend is not None:
                    pdist, pw0, pgs = pend
                    junk = work.tile([VPT, GR, NPTS], F32, tag="junk")
                    for j in range(pgs):
                        w = pw0 + j
                        nc.vector.tensor_scalar(
                            junk[:, j], pdist[:, j], 0.0, None, op0=ALU.add,
                            op1=ALU.add, accum_out=NUM[:, w : w + 1],
                        )
                pend = (dist, w0, gs)
            if pend is not None:
                pdist, pw0, pgs = pend
                junk = work.tile([VPT, GR, NPTS], F32, tag="junk")
                for j in range(pgs):
                    w = pw0 + j
                    nc.vector.tensor_scalar(
                        junk[:, j], pdist[:, j], 0.0, None, op0=ALU.add,
                        op1=ALU.add, accum_out=NUM[:, w : w + 1],
                    )
            t0 += bc

        nc.sync.dma_start(out=num_o[:, :], in_=NUM)
        nc.sync.dma_start(out=den_o[:, :], in_=DEN)

    nc.compile()
    return nc, nt


_CACHE = {}


def _get_nc(v_core=V_CORE):
    key = v_core
    if key not in _CACHE:
        _CACHE[key] = build_nc(v_core)
    return _CACHE[key]


def make_xaug3(points_x):
    """Host-built [128, 408] streamed constant: block-diag [24, 408] of
    three [8, 136] xaug blocks, replicated in all 4 partition quadrants."""
    xa = np.zeros((8, 2 * NPTS), dtype=np.float32)
    xa[0:3, 0:NPTS] = points_x.T
    xa[3, 0:NPTS] = -1.0
    xa[4:7, NPTS:] = points_x.T
    xa[7, NPTS:] = -1.0
    xq = np.zeros((128, GR * 2 * NPTS), dtype=np.float32)
    for q in range(4):
        for b in range(GR):
            xq[32 * q + 8 * b : 32 * q + 8 * b + 8,
               2 * NPTS * b : 2 * NPTS * (b + 1)] = xa
    return xq


def host_finish(nums, dens, v_core, v_total):
    """Combine per-core [128, nt] num/den partials into the scalar loss."""
    total = 0.0
    for num, den in zip(nums, dens):
        nt = num.shape[1]
        lv = num.astype(np.float64) / den.astype(np.float64)
        for w in range(nt):
            valid = min(VPT, v_core - w * VPT)
            total += lv[:valid, w].sum()
    return np.float32(total / (float(v_total) * float(v_total)))


def build_in_maps(points_x, points_y, v_core):
    body = points_y[NPTS:]
    xq = make_xaug3(points_x)
    in_maps = []
    for c in range(N_CORES):
        shard = np.ascontiguousarray(
            body[c * v_core * NPTS : (c + 1) * v_core * NPTS]
        )
        in_maps.append({"y": shard, "xaug3": xq})
    return in_maps


def kernel(points_x, points_y):
    points_x = np.asarray(points_x, dtype=np.float32)
    points_y = np.asarray(points_y, dtype=np.float32)
    v_total = (points_y.shape[0] - NPTS) // NPTS
    v_core = v_total // N_CORES
    nc, nt = _get_nc(v_core)

    in_maps = build_in_maps(points_x, points_y, v_core)
    res = run_bass_kernel_spmd(nc, in_maps, list(range(N_CORES)))
    nums = [res.results[c]["num"] for c in range(N_CORES)]
    dens = [res.results[c]["den"] for c in range(N_CORES)]
    return host_finish(nums, dens, v_core, v_total)
